# revision 4
# baseline (speedup 1.0000x reference)
"""Causal multi-head attention on 8 NeuronCores (Trainium2, Bass/Tile).

Problem: B=2, S=2048, E=1024, H=16, D=64, causal MHA with QKV/out projections.

Sharding: core c = (batch b = c//4, head-group g = c%4 of 4 heads).
Each core: QKV projection for its 4 heads, flash-style causal attention,
partial output projection (its heads' rows of W_out). Host sums the 4
partials per batch and adds b_out (+ the folded-in v-bias contribution).

Layout: all activations kept transposed ([dim, seq]) so that:
  - scoresT[k, q] = (kT as weights).T @ qT   (keys on partitions -> the
    softmax denominator comes from an appended ones-column in the PV matmul)
  - PV: outT[d, q] = (v[s,d]|ones as weights).T @ expT[k, q]
  - outT is directly the lhsT of the output projection.
No transposes of probabilities anywhere. q/k biases are applied as
per-partition ACT bias during the PSUM->SBUF copy; the v bias is folded into
the host-side output bias (probs rows sum to 1).
"""

import numpy as np

import concourse.bass as bass
import concourse.mybir as mybir
import concourse.tile as tile
from concourse import bacc
from concourse.bass_utils import run_bass_kernel_spmd
from concourse.masks import make_identity

P = 128
S = 2048
E = 1024
H = 16
D = 64
N_CORES = 8
GROUPS = 4            # head-groups (tensor parallel dimension)
GH = H // GROUPS      # 4 heads per core
PAIRS = GH // 2       # 2 head pairs per core
GW = GH * D           # 256 projection columns per core
ET = E // P           # 8 contraction tiles
ST = S // P           # 16 seq tiles
QB = 512              # query-block (free dim of scoresT tiles)
NQB = S // QB         # 4
VW = D + 1            # v columns per head incl. ones-column (denominator)
MASK_VAL = -1e30

f32 = mybir.dt.float32
FT = mybir.ActivationFunctionType
ALU = mybir.AluOpType

_CACHE = {}


def _build_module():
    nc = bacc.Bacc(
        "TRN2", target_bir_lowering=False, debug=False, enable_asserts=False
    )
    xT = nc.dram_tensor("xT", [E, S], f32, kind="ExternalInput").ap()
    wq = nc.dram_tensor("wq", [E, GW], f32, kind="ExternalInput").ap()
    wk = nc.dram_tensor("wk", [E, GW], f32, kind="ExternalInput").ap()
    wv = nc.dram_tensor("wv", [E, GW], f32, kind="ExternalInput").ap()
    bq = nc.dram_tensor("bq", [GW], f32, kind="ExternalInput").ap()
    bk = nc.dram_tensor("bk", [GW], f32, kind="ExternalInput").ap()
    wo = nc.dram_tensor("wo", [GW, E], f32, kind="ExternalInput").ap()
    out = nc.dram_tensor("out", [S, E], f32, kind="ExternalOutput").ap()

    with tile.TileContext(nc) as tc:
        with (
            tc.tile_pool(name="const", bufs=1) as cpool,
            tc.tile_pool(name="wts", bufs=1) as wpool,
            tc.tile_pool(name="acts", bufs=1) as apool,
            tc.tile_pool(name="stream", bufs=4) as strm,
            tc.tile_pool(name="stage", bufs=4) as stg,
        ):
            # ---- constants / weights --------------------------------------
            ident = cpool.tile([P, P], f32, tag="ident")
            make_identity(nc, ident)

            # master causal mask [128, 896]: master[j, c] = 0 if c >= j+384
            # else MASK_VAL. Boundary tile b uses cols [384-128b, 896-128b).
            mask = cpool.tile([P, 896], f32, tag="mask")
            nc.gpsimd.memset(mask[:], 0.0)
            nc.gpsimd.affine_select(
                out=mask[:],
                in_=mask[:],
                compare_op=ALU.is_ge,
                fill=MASK_VAL,
                base=-384,
                pattern=[[1, 896]],
                channel_multiplier=-1,
            )

            ones64 = cpool.tile([1, D], f32, tag="ones64")
            nc.gpsimd.memset(ones64[:], 1.0)

            bq_sb = cpool.tile([P, PAIRS], f32, tag="bq")
            bk_sb = cpool.tile([P, PAIRS], f32, tag="bk")
            nc.sync.dma_start(out=bq_sb[:], in_=bq.rearrange("(t p) -> p t", p=P))
            nc.sync.dma_start(out=bk_sb[:], in_=bk.rearrange("(t p) -> p t", p=P))

            wq_sb = wpool.tile([P, ET, GW], f32, tag="wq")
            wk_sb = wpool.tile([P, ET, GW], f32, tag="wk")
            wv_sb = wpool.tile([P, ET, GW], f32, tag="wv")
            nc.sync.dma_start(out=wq_sb[:], in_=wq.rearrange("(t p) m -> p t m", p=P))
            nc.sync.dma_start(out=wk_sb[:], in_=wk.rearrange("(t p) m -> p t m", p=P))
            nc.sync.dma_start(out=wv_sb[:], in_=wv.rearrange("(t p) m -> p t m", p=P))

            wo_sb = wpool.tile([P, PAIRS, E], f32, tag="wo")
            nc.sync.dma_start(out=wo_sb[:], in_=wo.rearrange("(t p) n -> p t n", p=P))

            # ---- persistent activations -----------------------------------
            qT = apool.tile([P, PAIRS, S], f32, tag="qT")    # [2*64 d, pair, s]
            kT = apool.tile([P, PAIRS, S], f32, tag="kT")
            vT2 = apool.tile([P, PAIRS, S], f32, tag="vT2")  # pre-transpose v
            v_sb = apool.tile([P, ST, GH * VW], f32, tag="v")  # [s, st, h*(64+1)]
            attnT = apool.tile([P, PAIRS, S], f32, tag="attnT")

            # ones-columns of v (PV denominator trick)
            for st in range(ST):
                vcols = v_sb[:, st, :].rearrange("p (h c) -> p h c", h=GH)
                nc.gpsimd.memset(vcols[:, :, D : D + 1], 1.0)

            # ---- phase 1: QKV projections (transposed outputs) ------------
            with tc.tile_pool(name="ps_qkv", bufs=1, space="PSUM") as psq:
                for sc in range(NQB):
                    cols = slice(sc * QB, (sc + 1) * QB)
                    pq, pk, pv = [], [], []
                    for pr in range(PAIRS):
                        pq.append(psq.tile([P, QB], f32, tag=f"pq{pr}", bufs=1, name=f"pq{pr}"))
                        pk.append(psq.tile([P, QB], f32, tag=f"pk{pr}", bufs=1, name=f"pk{pr}"))
                        pv.append(psq.tile([P, QB], f32, tag=f"pv{pr}", bufs=1, name=f"pv{pr}"))
                    for t in range(ET):
                        xt = strm.tile([P, QB], f32, tag="xt")
                        nc.sync.dma_start(
                            out=xt[:], in_=xT[t * P : (t + 1) * P, cols]
                        )
                        for pr in range(PAIRS):
                            mcol = slice(pr * P, (pr + 1) * P)
                            nc.tensor.matmul(
                                pq[pr][:], wq_sb[:, t, mcol], xt[:],
                                start=(t == 0), stop=(t == ET - 1),
                            )
                            nc.tensor.matmul(
                                pk[pr][:], wk_sb[:, t, mcol], xt[:],
                                start=(t == 0), stop=(t == ET - 1),
                            )
                            nc.tensor.matmul(
                                pv[pr][:], wv_sb[:, t, mcol], xt[:],
                                start=(t == 0), stop=(t == ET - 1),
                            )
                    for pr in range(PAIRS):
                        nc.scalar.activation(
                            qT[:, pr, cols], pq[pr][:], FT.Identity,
                            bias=bq_sb[:, pr : pr + 1], scale=1.0,
                        )
                        nc.scalar.activation(
                            kT[:, pr, cols], pk[pr][:], FT.Identity,
                            bias=bk_sb[:, pr : pr + 1], scale=1.0,
                        )
                        nc.vector.tensor_copy(vT2[:, pr, cols], pv[pr][:])

                # ---- phase 2: transpose v into [s, d] layout --------------
                for pr in range(PAIRS):
                    for st in range(ST):
                        pt = psq.tile([P, P], f32, tag="ptr", bufs=2)
                        nc.tensor.transpose(
                            pt[:], vT2[:, pr, st * P : (st + 1) * P], ident[:]
                        )
                        dst = v_sb[:, st, :].rearrange("p (h c) -> p h c", h=GH)
                        nc.vector.tensor_copy(
                            dst[:, 2 * pr : 2 * pr + 2, 0:D],
                            pt.rearrange("p (h c) -> p h c", h=2),
                        )

            # ---- phase 3: causal attention --------------------------------
            with (
                tc.tile_pool(name="ps_sc", bufs=4, space="PSUM") as pssc,
                tc.tile_pool(name="ps_po", bufs=3, space="PSUM") as pspo,
                tc.tile_pool(name="ps_bc", bufs=1, space="PSUM") as psbc,
            ):
                for pr in range(PAIRS):
                    for qb in range(NQB):
                        qcols = slice(qb * QB, (qb + 1) * QB)
                        nkt = 4 * (qb + 1)
                        po = [
                            pspo.tile([P, QB], f32, tag="po", bufs=3, name=f"po{i}")
                            for i in range(2)
                        ]
                        for kt in range(nkt):
                            for hh in range(2):
                                rows = slice(hh * D, (hh + 1) * D)
                                ps = pssc.tile([P, QB], f32, tag="sc", bufs=4)
                                nc.tensor.matmul(
                                    ps[:],
                                    kT[rows, pr, kt * P : (kt + 1) * P],
                                    qT[rows, pr, qcols],
                                    start=True, stop=True,
                                    tile_position=(hh * D, 0),
                                )
                                if kt >= 4 * qb:
                                    b = kt - 4 * qb
                                    moff = 384 - 128 * b
                                    nc.vector.tensor_tensor(
                                        ps[:], ps[:],
                                        mask[:, moff : moff + QB], ALU.add,
                                    )
                                ex = stg.tile([P, QB], f32, tag="ex", bufs=4)
                                nc.scalar.activation(
                                    ex[:], ps[:], FT.Exp, scale=0.125
                                )
                                h = 2 * pr + hh
                                nc.tensor.matmul(
                                    po[hh][0:VW, :],
                                    v_sb[:, kt, h * VW : (h + 1) * VW],
                                    ex[:],
                                    start=(kt == 0), stop=(kt == nkt - 1),
                                )
                        for hh in range(2):
                            rec = stg.tile([1, QB], f32, tag="rec", bufs=2)
                            nc.vector.reciprocal(rec[:], po[hh][D : D + 1, :])
                            bc = psbc.tile([D, QB], f32, tag="bc", bufs=1)
                            nc.tensor.matmul(
                                bc[:], ones64[:], rec[:], start=True, stop=True
                            )
                            bc_sb = stg.tile([D, QB], f32, tag="bcs", bufs=2)
                            nc.scalar.activation(bc_sb[:], bc[:], FT.Copy)
                            if hh == 0:
                                nc.vector.tensor_tensor(
                                    attnT[0:D, pr, qcols],
                                    po[hh][0:D, :], bc_sb[:], ALU.mult,
                                )
                            else:
                                bounce = stg.tile([D, QB], f32, tag="bn", bufs=2)
                                nc.vector.tensor_tensor(
                                    bounce[:], po[hh][0:D, :], bc_sb[:], ALU.mult
                                )
                                nc.sync.dma_start(
                                    out=attnT[D:P, pr, qcols], in_=bounce[:]
                                )

                # ---- phase 4: output projection (partial) -----------------
                for st in range(ST):
                    rows = slice(st * P, (st + 1) * P)
                    o_sb = stg.tile([P, E], f32, tag="osb", bufs=3)
                    for ch in range(2):
                        ncol = slice(ch * QB, (ch + 1) * QB)
                        pp = pssc.tile([P, QB], f32, tag="sc", bufs=4)
                        for pr in range(PAIRS):
                            nc.tensor.matmul(
                                pp[:],
                                attnT[:, pr, rows],
                                wo_sb[:, pr, ncol],
                                start=(pr == 0), stop=(pr == PAIRS - 1),
                            )
                        nc.vector.tensor_copy(o_sb[:, ncol], pp[:])
                    nc.sync.dma_start(out=out[rows, :], in_=o_sb[:])

    nc.compile()
    return nc


def _get_module():
    if "nc" not in _CACHE:
        _CACHE["nc"] = _build_module()
    return _CACHE["nc"]


def _make_in_maps(x, W_qkv, b_qkv, W_out):
    in_maps = []
    for c in range(N_CORES):
        b, g = divmod(c, GROUPS)
        cs = slice(g * GW, (g + 1) * GW)
        in_maps.append({
            "xT": np.ascontiguousarray(x[b].T),
            "wq": np.ascontiguousarray(W_qkv[:, 0 * E :][:, cs]),
            "wk": np.ascontiguousarray(W_qkv[:, 1 * E :][:, cs]),
            "wv": np.ascontiguousarray(W_qkv[:, 2 * E :][:, cs]),
            "bq": np.ascontiguousarray(b_qkv[0 * E :][cs]),
            "bk": np.ascontiguousarray(b_qkv[1 * E :][cs]),
            "wo": np.ascontiguousarray(W_out[g * GW : (g + 1) * GW, :]),
        })
    return in_maps


def kernel(x, W_qkv, b_qkv, W_out, b_out):
    x = np.asarray(x, dtype=np.float32)
    W_qkv = np.asarray(W_qkv, dtype=np.float32)
    b_qkv = np.asarray(b_qkv, dtype=np.float32)
    W_out = np.asarray(W_out, dtype=np.float32)
    b_out = np.asarray(b_out, dtype=np.float32)

    nc = _get_module()
    in_maps = _make_in_maps(x, W_qkv, b_qkv, W_out)
    res = run_bass_kernel_spmd(nc, in_maps, core_ids=list(range(N_CORES)))

    # v-bias folded here: attn rows of probs sum to 1 -> +b_v@W_out.
    bias = b_out + b_qkv[2 * E :] @ W_out
    B = 2
    full = np.empty((B, S, E), dtype=np.float32)
    for b in range(B):
        acc = res.results[4 * b]["out"].copy()
        for g in range(1, GROUPS):
            acc += res.results[4 * b + g]["out"]
        full[b] = acc + bias
    return full


# revision 10
# speedup vs baseline: 1.0692x; 1.0692x over previous
"""Causal multi-head attention on 8 NeuronCores (Trainium2, Bass/Tile).

Problem: B=2, S=2048, E=1024, H=16, D=64, causal MHA with QKV/out projections.

Sharding: core c = (batch b = c//4, head-group g = c%4 of 4 heads).
Each core: QKV projection for its 4 heads, flash-style causal attention,
partial output projection (its heads' rows of W_out). Host sums the 4
partials per batch and adds b_out (+ the folded-in v-bias contribution).

Layout: all activations kept transposed ([dim, seq]) so that:
  - scoresT[k, q] = (kT as weights).T @ qT   (keys on partitions -> the
    softmax denominator comes from an appended ones-column in the PV matmul)
  - PV: outT[d, q] = (v[s,d]|ones as weights).T @ expT[k, q]
  - outT is directly the lhsT of the output projection.
No transposes of probabilities anywhere. q/k biases are applied as
per-partition ACT bias during the PSUM->SBUF copy; the v bias is folded into
the host-side output bias (probs rows sum to 1).
"""

import numpy as np

import concourse.bass as bass
import concourse.mybir as mybir
import concourse.tile as tile
from concourse import bacc
from concourse.bass_utils import run_bass_kernel_spmd
from concourse.masks import make_identity

P = 128
S = 2048
E = 1024
H = 16
D = 64
N_CORES = 8
GROUPS = 4            # head-groups (tensor parallel dimension)
GH = H // GROUPS      # 4 heads per core
PAIRS = GH // 2       # 2 head pairs per core
GW = GH * D           # 256 projection columns per core
ET = E // P           # 8 contraction tiles
ST = S // P           # 16 seq tiles
QB = 512              # query-block (free dim of scoresT tiles)
NQB = S // QB         # 4
VW = D + 1            # v columns per head incl. ones-column (denominator)
MASK_VAL = -1e30

f32 = mybir.dt.float32
f32r = mybir.dt.float32r
FT = mybir.ActivationFunctionType
ALU = mybir.AluOpType

_CACHE = {}


def _build_module():
    nc = bacc.Bacc(
        "TRN2", target_bir_lowering=False, debug=False, enable_asserts=False
    )
    xT = nc.dram_tensor("xT", [E, S], f32r, kind="ExternalInput").ap()
    wq = nc.dram_tensor("wq", [E, GW], f32r, kind="ExternalInput").ap()
    wk = nc.dram_tensor("wk", [E, GW], f32r, kind="ExternalInput").ap()
    wv = nc.dram_tensor("wv", [E, GW], f32r, kind="ExternalInput").ap()
    bq = nc.dram_tensor("bq", [GW], f32, kind="ExternalInput").ap()
    bk = nc.dram_tensor("bk", [GW], f32, kind="ExternalInput").ap()
    wo = nc.dram_tensor("wo", [GW, E], f32r, kind="ExternalInput").ap()
    out = nc.dram_tensor("out", [S, E], f32, kind="ExternalOutput").ap()

    with tile.TileContext(nc) as tc:
        with (
            tc.tile_pool(name="const", bufs=1) as cpool,
            tc.tile_pool(name="wts", bufs=1) as wpool,
            tc.tile_pool(name="acts", bufs=1) as apool,
            tc.tile_pool(name="stream", bufs=4) as strm,
            tc.tile_pool(name="stage", bufs=4) as stg,
        ):
            # ---- constants / weights --------------------------------------
            ident_f = cpool.tile([P, P], f32, tag="ident_f")
            make_identity(nc, ident_f)
            ident = cpool.tile([P, P], f32r, tag="ident")
            nc.vector.tensor_copy(ident[:], ident_f[:])

            # master causal mask [128, 896]: master[j, c] = 0 if c >= j+384
            # else MASK_VAL. Boundary tile b uses cols [384-128b, 896-128b).
            mask = cpool.tile([P, 896], f32, tag="mask")
            nc.gpsimd.memset(mask[:], 0.0)
            nc.gpsimd.affine_select(
                out=mask[:],
                in_=mask[:],
                compare_op=ALU.is_ge,
                fill=MASK_VAL,
                base=-384,
                pattern=[[1, 896]],
                channel_multiplier=-1,
            )

            ones64 = cpool.tile([1, D], f32, tag="ones64")
            nc.gpsimd.memset(ones64[:], 1.0)
            onescol = cpool.tile([P, 1], f32, tag="onescol")
            nc.gpsimd.memset(onescol[:], 1.0)

            bq_sb = cpool.tile([P, PAIRS], f32, tag="bq")
            bk_sb = cpool.tile([P, PAIRS], f32, tag="bk")
            nc.sync.dma_start(out=bq_sb[:], in_=bq.rearrange("(t p) -> p t", p=P))
            nc.sync.dma_start(out=bk_sb[:], in_=bk.rearrange("(t p) -> p t", p=P))

            wq_sb = wpool.tile([P, ET, GW], f32r, tag="wq")
            wk_sb = wpool.tile([P, ET, GW], f32r, tag="wk")
            wv_sb = wpool.tile([P, ET, GW], f32r, tag="wv")
            nc.sync.dma_start(out=wq_sb[:], in_=wq.rearrange("(t p) m -> p t m", p=P))
            nc.sync.dma_start(out=wk_sb[:], in_=wk.rearrange("(t p) m -> p t m", p=P))
            nc.sync.dma_start(out=wv_sb[:], in_=wv.rearrange("(t p) m -> p t m", p=P))

            wo_sb = wpool.tile([P, PAIRS, E], f32r, tag="wo")
            nc.sync.dma_start(out=wo_sb[:], in_=wo.rearrange("(t p) n -> p t n", p=P))

            # ---- persistent activations -----------------------------------
            qT = apool.tile([P, PAIRS, S], f32r, tag="qT")    # [2*64 d, pair, s]
            kT = apool.tile([P, PAIRS, S], f32r, tag="kT")
            vT2 = apool.tile([P, PAIRS, S], f32r, tag="vT2")  # pre-transpose v
            v_sb = apool.tile([P, ST, GH * VW], f32r, tag="v")  # [s, st, h*(64+1)]
            attnT = apool.tile([P, PAIRS, S], f32r, tag="attnT")

            # ones-columns of v (PV denominator trick); DVE copy rounds
            # f32 -> f32r (memset can't write f32r directly)
            for st in range(ST):
                vcols = v_sb[:, st, :].rearrange("p (h c) -> p h c", h=GH)
                for h in range(GH):
                    nc.vector.tensor_copy(
                        vcols[:, h, D : D + 1], onescol[:]
                    )

            # ---- phase 1: QKV projections (transposed outputs) ------------
            with tc.tile_pool(name="ps_qkv", bufs=1, space="PSUM") as psq:
                for sc in range(NQB):
                    cols = slice(sc * QB, (sc + 1) * QB)
                    pq, pk, pv = [], [], []
                    for pr in range(PAIRS):
                        pq.append(psq.tile([P, QB], f32, tag=f"pq{pr}", bufs=1, name=f"pq{pr}"))
                        pk.append(psq.tile([P, QB], f32, tag=f"pk{pr}", bufs=1, name=f"pk{pr}"))
                        pv.append(psq.tile([P, QB], f32, tag=f"pv{pr}", bufs=1, name=f"pv{pr}"))
                    for t in range(ET):
                        xt = strm.tile([P, QB], f32r, tag="xt")
                        nc.sync.dma_start(
                            out=xt[:], in_=xT[t * P : (t + 1) * P, cols]
                        )
                        for pr in range(PAIRS):
                            mcol = slice(pr * P, (pr + 1) * P)
                            nc.tensor.matmul(
                                pq[pr][:], wq_sb[:, t, mcol], xt[:],
                                start=(t == 0), stop=(t == ET - 1),
                            )
                            nc.tensor.matmul(
                                pk[pr][:], wk_sb[:, t, mcol], xt[:],
                                start=(t == 0), stop=(t == ET - 1),
                            )
                            nc.tensor.matmul(
                                pv[pr][:], wv_sb[:, t, mcol], xt[:],
                                start=(t == 0), stop=(t == ET - 1),
                            )
                    for pr in range(PAIRS):
                        nc.scalar.activation(
                            qT[:, pr, cols], pq[pr][:], FT.Identity,
                            bias=bq_sb[:, pr : pr + 1], scale=1.0,
                        )
                        nc.scalar.activation(
                            kT[:, pr, cols], pk[pr][:], FT.Identity,
                            bias=bk_sb[:, pr : pr + 1], scale=1.0,
                        )
                        nc.vector.tensor_copy(vT2[:, pr, cols], pv[pr][:])

                # ---- phase 2: transpose v into [s, d] layout --------------
                for pr in range(PAIRS):
                    for st in range(ST):
                        pt = psq.tile([P, P], f32r, tag="ptr", bufs=2)
                        nc.tensor.transpose(
                            pt[:], vT2[:, pr, st * P : (st + 1) * P], ident[:]
                        )
                        dst = v_sb[:, st, :].rearrange("p (h c) -> p h c", h=GH)
                        nc.vector.tensor_copy(
                            dst[:, 2 * pr : 2 * pr + 2, 0:D],
                            pt.rearrange("p (h c) -> p h c", h=2),
                        )

            # ---- phase 3: causal attention --------------------------------
            with (
                tc.tile_pool(name="ps_sc", bufs=4, space="PSUM") as pssc,
                tc.tile_pool(name="ps_po", bufs=3, space="PSUM") as pspo,
                tc.tile_pool(name="ps_bc", bufs=1, space="PSUM") as psbc,
            ):
                for pr in range(PAIRS):
                    for qb in range(NQB):
                        qcols = slice(qb * QB, (qb + 1) * QB)
                        nkt = 4 * (qb + 1)
                        po = [
                            pspo.tile([P, QB], f32, tag="po", bufs=3, name=f"po{i}")
                            for i in range(2)
                        ]
                        for kt in range(nkt):
                            for hh in range(2):
                                rows = slice(hh * D, (hh + 1) * D)
                                ps = pssc.tile([P, QB], f32, tag="sc", bufs=4)
                                nc.tensor.matmul(
                                    ps[:],
                                    kT[rows, pr, kt * P : (kt + 1) * P],
                                    qT[rows, pr, qcols],
                                    start=True, stop=True,
                                    tile_position=(hh * D, 0),
                                )
                                if kt >= 4 * qb:
                                    b = kt - 4 * qb
                                    moff = 384 - 128 * b
                                    nc.vector.tensor_tensor(
                                        ps[:], ps[:],
                                        mask[:, moff : moff + QB], ALU.add,
                                    )
                                ex = stg.tile([P, QB], f32r, tag="ex", bufs=4)
                                nc.scalar.activation(
                                    ex[:], ps[:], FT.Exp, scale=0.125
                                )
                                h = 2 * pr + hh
                                nc.tensor.matmul(
                                    po[hh][0:VW, :],
                                    v_sb[:, kt, h * VW : (h + 1) * VW],
                                    ex[:],
                                    start=(kt == 0), stop=(kt == nkt - 1),
                                )
                        for hh in range(2):
                            rec = stg.tile([1, QB], f32, tag="rec", bufs=2)
                            nc.vector.reciprocal(rec[:], po[hh][D : D + 1, :])
                            bc = psbc.tile([D, QB], f32, tag="bc", bufs=1)
                            nc.tensor.matmul(
                                bc[:], ones64[:], rec[:], start=True, stop=True
                            )
                            bc_sb = stg.tile([D, QB], f32, tag="bcs", bufs=2)
                            nc.scalar.activation(bc_sb[:], bc[:], FT.Copy)
                            if hh == 0:
                                nc.vector.tensor_tensor(
                                    attnT[0:D, pr, qcols],
                                    po[hh][0:D, :], bc_sb[:], ALU.mult,
                                )
                            else:
                                bounce = stg.tile([D, QB], f32r, tag="bn", bufs=2)
                                nc.vector.tensor_tensor(
                                    bounce[:], po[hh][0:D, :], bc_sb[:], ALU.mult
                                )
                                nc.sync.dma_start(
                                    out=attnT[D:P, pr, qcols], in_=bounce[:]
                                )

                # ---- phase 4: output projection (partial) -----------------
                for st in range(ST):
                    rows = slice(st * P, (st + 1) * P)
                    o_sb = stg.tile([P, E], f32, tag="osb", bufs=3)
                    for ch in range(2):
                        ncol = slice(ch * QB, (ch + 1) * QB)
                        pp = pssc.tile([P, QB], f32, tag="sc", bufs=4)
                        for pr in range(PAIRS):
                            nc.tensor.matmul(
                                pp[:],
                                attnT[:, pr, rows],
                                wo_sb[:, pr, ncol],
                                start=(pr == 0), stop=(pr == PAIRS - 1),
                            )
                        nc.vector.tensor_copy(o_sb[:, ncol], pp[:])
                    nc.sync.dma_start(out=out[rows, :], in_=o_sb[:])

    nc.compile()
    return nc


def _get_module():
    if "nc" not in _CACHE:
        _CACHE["nc"] = _build_module()
    return _CACHE["nc"]


def _round_f32r(a):
    """Round-to-nearest-even to fp32r precision (11 mantissa bits) --
    bit-exact match of the device's fp32r rounding, so DMA'd operands are
    already exactly representable."""
    a = np.ascontiguousarray(a, dtype=np.float32)
    u = a.view(np.uint32)
    lsb = (u >> 12) & np.uint32(1)
    r = (u + np.uint32(0x7FF) + lsb) & np.uint32(0xFFFFF000)
    return r.view(np.float32)


def _make_in_maps(x, W_qkv, b_qkv, W_out):
    in_maps = []
    for c in range(N_CORES):
        b, g = divmod(c, GROUPS)
        cs = slice(g * GW, (g + 1) * GW)
        in_maps.append({
            "xT": _round_f32r(x[b].T),
            "wq": _round_f32r(W_qkv[:, 0 * E :][:, cs]),
            "wk": _round_f32r(W_qkv[:, 1 * E :][:, cs]),
            "wv": _round_f32r(W_qkv[:, 2 * E :][:, cs]),
            "bq": np.ascontiguousarray(b_qkv[0 * E :][cs]),
            "bk": np.ascontiguousarray(b_qkv[1 * E :][cs]),
            "wo": _round_f32r(W_out[g * GW : (g + 1) * GW, :]),
        })
    return in_maps


def kernel(x, W_qkv, b_qkv, W_out, b_out):
    x = np.asarray(x, dtype=np.float32)
    W_qkv = np.asarray(W_qkv, dtype=np.float32)
    b_qkv = np.asarray(b_qkv, dtype=np.float32)
    W_out = np.asarray(W_out, dtype=np.float32)
    b_out = np.asarray(b_out, dtype=np.float32)

    nc = _get_module()
    in_maps = _make_in_maps(x, W_qkv, b_qkv, W_out)
    res = run_bass_kernel_spmd(nc, in_maps, core_ids=list(range(N_CORES)))

    # v-bias folded here: attn rows of probs sum to 1 -> +b_v@W_out.
    bias = b_out + b_qkv[2 * E :] @ W_out
    B = 2
    full = np.empty((B, S, E), dtype=np.float32)
    for b in range(B):
        acc = res.results[4 * b]["out"].copy()
        for g in range(1, GROUPS):
            acc += res.results[4 * b + g]["out"]
        full[b] = acc + bias
    return full
